# revision 22
# baseline (speedup 1.0000x reference)
"""Trainium2 Bass kernel for nn_Attention (general-score attention energies +
softmax over the batch axis).

Math (reference):
    proj     = einsum('lbh,oh->lbo', enc, W) + b      # [L, B, H]
    energies = einsum('bh,lbh->bl', hidden, proj)     # [B, L]
    attn     = softmax(energies, axis=0)[:, None, :]  # [B, 1, L]

Algebraic rewrite used here:
    energies[b, l] = (hidden @ W)[b] . enc[l, b] + hidden[b] . b
This removes the O(L*B*H*H) projection matmul entirely; the kernel is a
memory-bound stream over enc with a tiny [B,H]x[H,H] matmul up front.

fp16 strategy: enc / W / hidden are cast to fp16 on the host (pure dtype
compression, all FLOPs stay on device; fp32 accumulation everywhere).
Measured end-to-end rel err vs the fp32 reference: ~1.8e-3 (gate: 2e-2).
This halves HBM traffic AND enables the DVE 2x_1P perf mode.

The dot-product stream is compute-floored by the 1024-wide row sums:
every reduce flavor (STT/tensor_scalar accum, tensor_reduce, bn_stats,
ACT ACTIVATE+accum) runs at 1 elem/cycle/lane (~1.2-1.3us per [128,1024]
block); only the plain fp16 tensor_tensor multiply has a 2x mode
(~570ns/block grouped). GpSimd cannot run the accum ops at all (walrus
engine check) and contends with DVE for SBUF ports, so the optimal
schedule uses DVE+ACT only:
  - 44 "A" blocks: DVE grouped TT multiply (in place over the streamed
    tile, vs a stride-0-broadcast view of u) + ScalarE ACTIVATE(Copy)
    with fused accum_out for the row sum.
  - 20 "D" blocks: one fused DVE scalar_tensor_tensor (1x) does
    multiply+sum in a single pass.
Both engines land at ~56us of stream work, overlapping the ~50us DMA.

Distribution: enc is sharded along L across 8 cores (128 l-values per
core). The softmax is over the batch axis (per l), so every core's
softmax is fully local -- no collectives. hidden / W / b are replicated.

Setup path: W fp16 in four 1 MB k-major quarter DMAs; the 16 PE matmuls
for u = hidden @ W run k-outer so each k-chunk is consumed as it lands.
hidden^T arrives pre-transposed and pair-duplicated so the matmul output
covers all 128 PSUM partitions. The big enc-tile DMAs are data-gated
behind u16 (sliver copies from u16 into each buffer) so only the four
small leading tiles compete with W for DMA bandwidth during setup.
Softmax runs in two column halves so the first half overlaps the
stream; output leaves in two [64,64] DMAs.

Timing (HW, neuron-profile, core 0): 86-91 us cool, up to ~103 us when
the HAM activity throttle (50% util limit, engages ~20 us in) bites
harder on a thermally loaded device. fp32 baseline: 119-142 us.
Breakdown (cool): ~10 us NEFF/queue startup, stream start ~20 us
(W wire + PE chain; fp16 matmul streams at ~630ns/512 cols, ~3x the
bf16 rate), DVE+ACT balanced stream ~52-55 us each, ~5 us tail.
"""

import numpy as np

import concourse.bass as bass
import concourse.bacc as bacc
import concourse.tile as tile
from concourse import mybir
from concourse.bass_utils import run_bass_kernel_spmd

F32 = mybir.dt.float32
F16 = mybir.dt.float16

B = 64          # batch
H = 1024        # hidden dim
L = 1024        # enc_len
NCORES = 8
LS = L // NCORES            # 128 l-values per core
NBLK = LS * B // 128        # 64 [128, 1024] blocks per core
# blocks per DMA tile: small leading tiles so compute starts early
TILE_BLOCKS = [2, 2, 2, 2] + [8] * 7
assert sum(TILE_BLOCKS) == NBLK
# A-blocks (DVE mult + ACT reduce) per tile; the rest are fused-STT D-blocks
TILE_A = [1, 1, 1, 1, 5, 5, 5, 6, 6, 5, 6]   # 42 A / 22 D
MULT = mybir.AluOpType.mult
ADD = mybir.AluOpType.add


def build_program() -> bacc.Bacc:
    nc = bacc.Bacc(
        "TRN2", target_bir_lowering=False, debug=False, num_devices=NCORES
    )

    setup16_p = nc.declare_dram_parameter("setup16", [128, 1032], F16, isOutput=False)
    setup32_p = nc.declare_dram_parameter("setup32", [128, 128], F32, isOutput=False)
    enc_p = nc.declare_dram_parameter("enc", [LS * B, H], F16, isOutput=False)
    w_p = nc.declare_dram_parameter("w", [H, H], F16, isOutput=False)
    out_p = nc.declare_dram_parameter("out", [B, LS], F32, isOutput=True)

    # NOTE: must be built as bacc.Bacc + nc.compile() -- the staged walrus
    # rejects multi-wait instructions emitted by raw Bass+Tile; bacc
    # legalizes them.
    with tile.TileContext(nc) as tc:
        with (
            tc.tile_pool(name="const", bufs=1) as cp,
            tc.tile_pool(name="stream", bufs=6) as sp,
            tc.tile_pool(name="early", bufs=4) as spe,
            tc.tile_pool(name="ps1", bufs=1, space="PSUM") as pp1,
            tc.tile_pool(name="psu", bufs=1, space="PSUM") as ppu,
        ):
            # ---- input DMAs (setup on the ACT ring so it does not queue
            # behind W/enc on the SP ring) ----
            setup16 = cp.tile([128, 1032], F16)
            setup32 = cp.tile([128, 128], F32)
            nc.scalar.dma_start(setup16[:], setup16_p.ap())
            nc.scalar.dma_start(setup32[:], setup32_p.ap())
            hT2 = setup16[:, 0:1024]      # chunk k at [:, 128k:128k+128]
            bvT = setup16[:, 1024:1032]
            idn = setup32

            # W as [o%128, (o//128, h)] fp16, four 1 MB k-major quarters so
            # the k-outer matmul chain consumes chunks as they land
            wt = cp.tile([128, 8 * H], F16)
            wt3 = wt[:].rearrange("p (k h) -> p k h", k=8)
            wsrc = w_p.ap().rearrange("(k p) h -> p k h", p=128)
            for kh in range(4):
                nc.sync.dma_start(
                    wt3[:, 2 * kh : 2 * kh + 2, :],
                    wsrc[:, 2 * kh : 2 * kh + 2, :],
                )

            # ---- u = hidden @ W on both b-halves ([128, 1024] fp32 PSUM),
            # k-outer so chunk k is consumed as soon as its quarter lands
            psum_u = ppu.tile([128, H], F32, tag="psum_u")
            u16 = cp.tile([128, H], F16)
            for k in range(8):
                for n in range(2):
                    nc.tensor.matmul(
                        psum_u[:, 512 * n : 512 * (n + 1)],
                        lhsT=hT2[:, 128 * k : 128 * (k + 1)],
                        rhs=wt[:, 1024 * k + 512 * n : 1024 * k + 512 * n + 512],
                        start=(k == 0),
                        stop=(k == 7),
                    )
            # one PSUM->SBUF half-copy on each engine, in parallel
            nc.scalar.copy(u16[:, 0:512], psum_u[:, 0:512])
            nc.vector.tensor_copy(u16[:, 512:1024], psum_u[:, 512:1024])
            u16b = u16[:].rearrange("p (x h) -> p x h", x=1)

            # ---- c[b] = hidden[b] . bvec (both b-halves via dup'd hT2) ----
            psum_c = ppu.tile([128, 1], F32, tag="psum_c")
            for k in range(8):
                nc.tensor.matmul(
                    psum_c[:],
                    lhsT=hT2[:, 128 * k : 128 * (k + 1)],
                    rhs=bvT[:, k : k + 1],
                    start=(k == 0),
                    stop=(k == 7),
                )
            c2 = cp.tile([128, 1], F32)
            nc.scalar.copy(c2[:], psum_c[:])

            # ---- main stream ----
            enc_flat = enc_p.ap()  # [8192, 1024] fp16
            ecols = cp.tile([128, NBLK], F32)  # [128, 64]
            ecols2 = cp.tile([128, NBLK], F32)
            out_sb = cp.tile([B, LS], F32)
            ov = out_sb[:].rearrange("b (l two) -> b two l", two=2)

            def softmax_half(hf):
                # bias, then softmax over b for columns [32hf, 32hf+32)
                # (l-values [64hf, 64hf+64)), both l-parities
                cs = slice(32 * hf, 32 * hf + 32)
                nc.vector.tensor_scalar_add(
                    ecols2[:, cs], ecols[:, cs], c2[:, 0:1]
                )
                for lp in range(2):
                    psum_t = pp1.tile([32, B], F32, tag=f"pt{lp}")
                    nc.tensor.transpose(
                        psum_t[:],
                        ecols2[lp * B : (lp + 1) * B, cs],
                        idn[lp * B : (lp + 1) * B, lp * B : (lp + 1) * B],
                    )
                    negm = cp.tile([32, 1], F32, tag=f"negm{lp}")
                    nc.vector.tensor_reduce(
                        out=negm[:],
                        in_=psum_t[:],
                        axis=mybir.AxisListType.X,
                        op=mybir.AluOpType.max,
                        negate=True,
                    )
                    pexp = cp.tile([32, B], F32, tag=f"pexp{lp}")
                    ssum = cp.tile([32, 1], F32, tag=f"ssum{lp}")
                    nc.scalar.activation(
                        pexp[:],
                        psum_t[:],
                        mybir.ActivationFunctionType.Exp,
                        bias=negm[:, 0:1],
                        scale=1.0,
                        accum_out=ssum[:],
                    )
                    rs = cp.tile([32, 1], F32, tag=f"rs{lp}")
                    nc.vector.reciprocal(rs[:], ssum[:])
                    attn = cp.tile([32, B], F32, tag=f"attn{lp}")
                    nc.vector.tensor_scalar_mul(attn[:], pexp[:], rs[:, 0:1])
                    psum_o = pp1.tile([B, 32], F32, tag="po")
                    nc.tensor.transpose(psum_o[:], attn[:], idn[0:32, 0:32])
                    nc.vector.tensor_copy(ov[:, lp, cs], psum_o[:])
                # contiguous l-range [64hf, 64hf+64) covers both parities
                nc.sync.dma_start(
                    out_p.ap()[:, 64 * hf : 64 * hf + 64],
                    out_sb[:, 64 * hf : 64 * hf + 64],
                )

            # DMA gating: the big-tile DMAs are data-gated behind u16
            # (sliver copies) and the small early tiles behind the last W
            # quarter (sliver from wt's final column) so W never competes
            # with enc for DMA bandwidth during the setup phase
            n_big = sum(1 for x in TILE_BLOCKS if x == 8)
            n_early = len(TILE_BLOCKS) - n_big
            bigs = [
                sp.tile([128, 8 * H], F16, tag="et", name=f"et{i}")
                for i in range(n_big)
            ]
            earls = [
                spe.tile([128, 2 * H], F16, tag="ete", name=f"ete{i}")
                for i in range(n_early)
            ]
            for et in earls:
                nc.vector.tensor_copy(et[:, 0:1], wt[:, 8 * H - 1 : 8 * H])
            for et in bigs[: 6]:
                nc.vector.tensor_copy(et[:, 0:1], u16[:, 0:1])
            bi = 0
            ei = 0
            c0 = 0
            for t, nq in enumerate(TILE_BLOCKS):
                na = TILE_A[t]
                if nq == 8:
                    et = bigs[bi]
                    bi += 1
                else:
                    et = earls[ei]
                    ei += 1
                src = enc_flat[128 * c0 : 128 * (c0 + nq)].rearrange(
                    "(q p) h -> p q h", p=128
                )
                nc.sync.dma_start(
                    et[:, 0 : nq * H].rearrange("p (q h) -> p q h", q=nq), src
                )
                # A-prefix in sub-groups of <=3: the DVE multiply (fp16 2x,
                # in place) releases each sub-group to ACT sooner, cutting
                # ACT's phase-wait at tile boundaries
                g0 = 0
                while g0 < na:
                    gn = min(3, na - g0)
                    nc.vector.tensor_tensor(
                        out=et[:, H * g0 : H * (g0 + gn)].rearrange(
                            "p (g h) -> p g h", g=gn
                        ),
                        in0=et[:, H * g0 : H * (g0 + gn)].rearrange(
                            "p (g h) -> p g h", g=gn
                        ),
                        in1=u16b.broadcast_to((128, gn, H)),
                        op=MULT,
                    )
                    for q in range(g0, g0 + gn):
                        c = c0 + q
                        blk = et[:, H * q : H * (q + 1)]
                        nc.scalar.activation(
                            blk,
                            blk,
                            mybir.ActivationFunctionType.Copy,
                            bias=0.0,
                            scale=1.0,
                            accum_out=ecols[:, c : c + 1],
                        )
                    g0 += gn
                # D-suffix: fused multiply+sum on DVE (1x STT)
                for q in range(na, nq):
                    c = c0 + q
                    blk = et[:, H * q : H * (q + 1)]
                    nc.vector.scalar_tensor_tensor(
                        out=blk,
                        in0=blk,
                        scalar=1.0,
                        in1=u16[:],
                        op0=MULT,
                        op1=MULT,
                        accum_out=ecols[:, c : c + 1],
                    )
                c0 += nq
                if c0 == 32:
                    softmax_half(0)
            softmax_half(1)

    nc.compile()
    return nc


_IDENT = np.eye(128, dtype=np.float32)
_NC_CACHE = []


def _get_nc() -> bacc.Bacc:
    if not _NC_CACHE:
        _NC_CACHE.append(build_program())
    return _NC_CACHE[0]


def make_in_maps(hidden, encoder_outputs, W, b):
    # host side does only layout transforms + fp16 dtype compression; all
    # FLOPs stay on device (fp32 accumulation)
    hidden = np.asarray(hidden, dtype=np.float32)
    W16 = np.ascontiguousarray(np.asarray(W, dtype=np.float32).astype(np.float16))
    hidT2 = np.concatenate([hidden.T, hidden.T], axis=1)  # [H, 2B]
    # chunk k rows -> [p, (k, m)] so setup16[:, 128k:128k+128] is lhsT chunk k
    hidT2p = hidT2.reshape(8, 128, 128).transpose(1, 0, 2).reshape(128, 1024)
    bvT = np.asarray(b, dtype=np.float32).reshape(8, 128).T  # [128, 8]
    setup16 = np.ascontiguousarray(
        np.concatenate([hidT2p, bvT], axis=1).astype(np.float16)
    )
    setup32 = _IDENT
    in_maps = []
    for i in range(NCORES):
        shard = (
            np.asarray(encoder_outputs[i * LS : (i + 1) * LS], dtype=np.float32)
            .astype(np.float16)
            .reshape(LS * B, H)
        )
        in_maps.append(
            {
                "setup16": setup16,
                "setup32": setup32,
                "enc": np.ascontiguousarray(shard),
                "w": W16,
            }
        )
    return in_maps


def kernel(hidden, encoder_outputs, W, b):
    nc = _get_nc()
    in_maps = make_in_maps(hidden, encoder_outputs, W, b)
    res = run_bass_kernel_spmd(nc, in_maps, core_ids=list(range(NCORES)))
    out = np.concatenate([res.results[i]["out"] for i in range(NCORES)], axis=1)
    return out[:, None, :].astype(np.float32)


# revision 23
# speedup vs baseline: 1.0173x; 1.0173x over previous
"""Trainium2 Bass kernel for nn_Attention (general-score attention energies +
softmax over the batch axis).

Math (reference):
    proj     = einsum('lbh,oh->lbo', enc, W) + b      # [L, B, H]
    energies = einsum('bh,lbh->bl', hidden, proj)     # [B, L]
    attn     = softmax(energies, axis=0)[:, None, :]  # [B, 1, L]

Algebraic rewrite used here:
    energies[b, l] = (hidden @ W)[b] . enc[l, b] + hidden[b] . b
This removes the O(L*B*H*H) projection matmul entirely; the kernel is a
memory-bound stream over enc with a tiny [B,H]x[H,H] matmul up front.

fp16 strategy: enc / W / hidden are cast to fp16 on the host (pure dtype
compression, all FLOPs stay on device; fp32 accumulation everywhere).
Measured end-to-end rel err vs the fp32 reference: ~1.8e-3 (gate: 2e-2).
This halves HBM traffic AND enables the DVE 2x_1P perf mode.

The dot-product stream is compute-floored by the 1024-wide row sums:
every reduce flavor (STT/tensor_scalar accum, tensor_reduce, bn_stats,
ACT ACTIVATE+accum) runs at 1 elem/cycle/lane (~1.2-1.3us per [128,1024]
block); only the plain fp16 tensor_tensor multiply has a 2x mode
(~570ns/block grouped). GpSimd cannot run the accum ops at all (walrus
engine check) and contends with DVE for SBUF ports, so the optimal
schedule uses DVE+ACT only:
  - 44 "A" blocks: DVE grouped TT multiply (in place over the streamed
    tile, vs a stride-0-broadcast view of u) + ScalarE ACTIVATE(Copy)
    with fused accum_out for the row sum.
  - 20 "D" blocks: one fused DVE scalar_tensor_tensor (1x) does
    multiply+sum in a single pass.
Both engines land at ~56us of stream work, overlapping the ~50us DMA.

Distribution: enc is sharded along L across 8 cores (128 l-values per
core). The softmax is over the batch axis (per l), so every core's
softmax is fully local -- no collectives. hidden / W / b are replicated.

Setup path: W fp16 in four 1 MB k-major quarter DMAs; the 16 PE matmuls
for u = hidden @ W run k-outer so each k-chunk is consumed as it lands.
hidden^T arrives pre-transposed and pair-duplicated so the matmul output
covers all 128 PSUM partitions. The big enc-tile DMAs are data-gated
behind u16 (sliver copies from u16 into each buffer) so only the four
small leading tiles compete with W for DMA bandwidth during setup.
Softmax runs in two column halves so the first half overlaps the
stream; output leaves in two [64,64] DMAs.

Timing (HW, neuron-profile, core 0): 86-91 us cool, up to ~103 us when
the HAM activity throttle (50% util limit, engages ~20 us in) bites
harder on a thermally loaded device. fp32 baseline: 119-142 us.
Breakdown (cool): ~10 us NEFF/queue startup, stream start ~20 us
(W wire + PE chain; fp16 matmul streams at ~630ns/512 cols, ~3x the
bf16 rate), DVE+ACT balanced stream ~52-55 us each, ~5 us tail.
"""

import numpy as np

import concourse.bass as bass
import concourse.bacc as bacc
import concourse.tile as tile
from concourse import mybir
from concourse.bass_utils import run_bass_kernel_spmd

F32 = mybir.dt.float32
F16 = mybir.dt.float16

B = 64          # batch
H = 1024        # hidden dim
L = 1024        # enc_len
NCORES = 8
LS = L // NCORES            # 128 l-values per core
NBLK = LS * B // 128        # 64 [128, 1024] blocks per core
# blocks per DMA tile: small leading tiles so compute starts early
TILE_BLOCKS = [2, 2, 2, 2] + [8] * 7
assert sum(TILE_BLOCKS) == NBLK
# A-blocks (DVE mult + ACT reduce) per tile; the rest are fused-STT D-blocks
TILE_A = [1, 1, 1, 1, 5, 5, 5, 6, 6, 5, 6]   # 42 A / 22 D
MULT = mybir.AluOpType.mult
ADD = mybir.AluOpType.add


def build_program() -> bacc.Bacc:
    nc = bacc.Bacc(
        "TRN2", target_bir_lowering=False, debug=False, num_devices=NCORES
    )

    setup16_p = nc.declare_dram_parameter("setup16", [128, 1032], F16, isOutput=False)
    setup32_p = nc.declare_dram_parameter("setup32", [128, 128], F32, isOutput=False)
    enc_p = nc.declare_dram_parameter("enc", [LS * B, H], F16, isOutput=False)
    w_p = nc.declare_dram_parameter("w", [H, H], F16, isOutput=False)
    out_p = nc.declare_dram_parameter("out", [B, LS], F32, isOutput=True)

    # NOTE: must be built as bacc.Bacc + nc.compile() -- the staged walrus
    # rejects multi-wait instructions emitted by raw Bass+Tile; bacc
    # legalizes them.
    with tile.TileContext(nc) as tc:
        with (
            tc.tile_pool(name="const", bufs=1) as cp,
            tc.tile_pool(name="stream", bufs=6) as sp,
            tc.tile_pool(name="early", bufs=4) as spe,
            tc.tile_pool(name="ps1", bufs=1, space="PSUM") as pp1,
            tc.tile_pool(name="psu", bufs=1, space="PSUM") as ppu,
        ):
            # ---- input DMAs (setup on the ACT ring so it does not queue
            # behind W/enc on the SP ring) ----
            setup16 = cp.tile([128, 1032], F16)
            setup32 = cp.tile([128, 128], F32)
            nc.scalar.dma_start(setup16[:], setup16_p.ap())
            nc.scalar.dma_start(setup32[:], setup32_p.ap())
            hT2 = setup16[:, 0:1024]      # chunk k at [:, 128k:128k+128]
            bvT = setup16[:, 1024:1032]
            idn = setup32

            # W as [o%128, (o//128, h)] fp16, four 1 MB k-major quarters so
            # the k-outer matmul chain consumes chunks as they land
            wt = cp.tile([128, 8 * H], F16)
            wt3 = wt[:].rearrange("p (k h) -> p k h", k=8)
            wsrc = w_p.ap().rearrange("(k p) h -> p k h", p=128)
            for kh in range(4):
                nc.sync.dma_start(
                    wt3[:, 2 * kh : 2 * kh + 2, :],
                    wsrc[:, 2 * kh : 2 * kh + 2, :],
                )

            # ---- u = hidden @ W on both b-halves ([128, 1024] fp32 PSUM),
            # k-outer so chunk k is consumed as soon as its quarter lands
            psum_u = ppu.tile([128, H], F32, tag="psum_u")
            u16 = cp.tile([128, H], F16)
            for k in range(8):
                for n in range(2):
                    nc.tensor.matmul(
                        psum_u[:, 512 * n : 512 * (n + 1)],
                        lhsT=hT2[:, 128 * k : 128 * (k + 1)],
                        rhs=wt[:, 1024 * k + 512 * n : 1024 * k + 512 * n + 512],
                        start=(k == 0),
                        stop=(k == 7),
                    )
            # one PSUM->SBUF half-copy on each engine, in parallel
            nc.scalar.copy(u16[:, 0:512], psum_u[:, 0:512])
            nc.vector.tensor_copy(u16[:, 512:1024], psum_u[:, 512:1024])
            u16b = u16[:].rearrange("p (x h) -> p x h", x=1)

            # ---- c[b] = hidden[b] . bvec (both b-halves via dup'd hT2) ----
            psum_c = ppu.tile([128, 1], F32, tag="psum_c")
            for k in range(8):
                nc.tensor.matmul(
                    psum_c[:],
                    lhsT=hT2[:, 128 * k : 128 * (k + 1)],
                    rhs=bvT[:, k : k + 1],
                    start=(k == 0),
                    stop=(k == 7),
                )
            c2 = cp.tile([128, 1], F32)
            nc.scalar.copy(c2[:], psum_c[:])

            # ---- main stream ----
            enc_flat = enc_p.ap()  # [8192, 1024] fp16
            ecols = cp.tile([128, NBLK], F32)  # [128, 64]
            ecols2 = cp.tile([128, NBLK], F32)
            out_sb = cp.tile([B, LS], F32)
            ov = out_sb[:].rearrange("b (l two) -> b two l", two=2)

            def softmax_half(hf):
                # bias, then softmax over b for columns [32hf, 32hf+32)
                # (l-values [64hf, 64hf+64)), both l-parities
                cs = slice(32 * hf, 32 * hf + 32)
                nc.vector.tensor_scalar_add(
                    ecols2[:, cs], ecols[:, cs], c2[:, 0:1]
                )
                for lp in range(2):
                    psum_t = pp1.tile([32, B], F32, tag=f"pt{lp}")
                    nc.tensor.transpose(
                        psum_t[:],
                        ecols2[lp * B : (lp + 1) * B, cs],
                        idn[lp * B : (lp + 1) * B, lp * B : (lp + 1) * B],
                    )
                    negm = cp.tile([32, 1], F32, tag=f"negm{lp}")
                    nc.vector.tensor_reduce(
                        out=negm[:],
                        in_=psum_t[:],
                        axis=mybir.AxisListType.X,
                        op=mybir.AluOpType.max,
                        negate=True,
                    )
                    pexp = cp.tile([32, B], F32, tag=f"pexp{lp}")
                    ssum = cp.tile([32, 1], F32, tag=f"ssum{lp}")
                    nc.scalar.activation(
                        pexp[:],
                        psum_t[:],
                        mybir.ActivationFunctionType.Exp,
                        bias=negm[:, 0:1],
                        scale=1.0,
                        accum_out=ssum[:],
                    )
                    rs = cp.tile([32, 1], F32, tag=f"rs{lp}")
                    nc.vector.reciprocal(rs[:], ssum[:])
                    attn = cp.tile([32, B], F32, tag=f"attn{lp}")
                    nc.vector.tensor_scalar_mul(attn[:], pexp[:], rs[:, 0:1])
                    psum_o = pp1.tile([B, 32], F32, tag="po")
                    nc.tensor.transpose(psum_o[:], attn[:], idn[0:32, 0:32])
                    nc.vector.tensor_copy(ov[:, lp, cs], psum_o[:])
                # contiguous l-range [64hf, 64hf+64) covers both parities
                nc.sync.dma_start(
                    out_p.ap()[:, 64 * hf : 64 * hf + 64],
                    out_sb[:, 64 * hf : 64 * hf + 64],
                )

            # DMA gating: the big-tile DMAs are data-gated behind u16
            # (sliver copies) and the small early tiles behind the last W
            # quarter (sliver from wt's final column) so W never competes
            # with enc for DMA bandwidth during the setup phase
            n_big = sum(1 for x in TILE_BLOCKS if x == 8)
            n_early = len(TILE_BLOCKS) - n_big
            bigs = [
                sp.tile([128, 8 * H], F16, tag="et", name=f"et{i}")
                for i in range(n_big)
            ]
            earls = [
                spe.tile([128, 2 * H], F16, tag="ete", name=f"ete{i}")
                for i in range(n_early)
            ]
            for et in bigs[: 6]:
                nc.vector.tensor_copy(et[:, 0:1], u16[:, 0:1])
            bi = 0
            ei = 0
            c0 = 0
            for t, nq in enumerate(TILE_BLOCKS):
                na = TILE_A[t]
                if nq == 8:
                    et = bigs[bi]
                    bi += 1
                else:
                    et = earls[ei]
                    ei += 1
                src = enc_flat[128 * c0 : 128 * (c0 + nq)].rearrange(
                    "(q p) h -> p q h", p=128
                )
                nc.sync.dma_start(
                    et[:, 0 : nq * H].rearrange("p (q h) -> p q h", q=nq), src
                )
                # A-prefix in sub-groups of <=3: the DVE multiply (fp16 2x,
                # in place) releases each sub-group to ACT sooner, cutting
                # ACT's phase-wait at tile boundaries
                g0 = 0
                while g0 < na:
                    gn = min(3, na - g0)
                    nc.vector.tensor_tensor(
                        out=et[:, H * g0 : H * (g0 + gn)].rearrange(
                            "p (g h) -> p g h", g=gn
                        ),
                        in0=et[:, H * g0 : H * (g0 + gn)].rearrange(
                            "p (g h) -> p g h", g=gn
                        ),
                        in1=u16b.broadcast_to((128, gn, H)),
                        op=MULT,
                    )
                    for q in range(g0, g0 + gn):
                        c = c0 + q
                        blk = et[:, H * q : H * (q + 1)]
                        nc.scalar.activation(
                            blk,
                            blk,
                            mybir.ActivationFunctionType.Copy,
                            bias=0.0,
                            scale=1.0,
                            accum_out=ecols[:, c : c + 1],
                        )
                    g0 += gn
                # D-suffix: fused multiply+sum on DVE (1x STT)
                for q in range(na, nq):
                    c = c0 + q
                    blk = et[:, H * q : H * (q + 1)]
                    nc.vector.scalar_tensor_tensor(
                        out=blk,
                        in0=blk,
                        scalar=1.0,
                        in1=u16[:],
                        op0=MULT,
                        op1=MULT,
                        accum_out=ecols[:, c : c + 1],
                    )
                c0 += nq
                if c0 == 32:
                    softmax_half(0)
            softmax_half(1)

    nc.compile()
    return nc


_IDENT = np.eye(128, dtype=np.float32)
_NC_CACHE = []


def _get_nc() -> bacc.Bacc:
    if not _NC_CACHE:
        _NC_CACHE.append(build_program())
    return _NC_CACHE[0]


def make_in_maps(hidden, encoder_outputs, W, b):
    # host side does only layout transforms + fp16 dtype compression; all
    # FLOPs stay on device (fp32 accumulation)
    hidden = np.asarray(hidden, dtype=np.float32)
    W16 = np.ascontiguousarray(np.asarray(W, dtype=np.float32).astype(np.float16))
    hidT2 = np.concatenate([hidden.T, hidden.T], axis=1)  # [H, 2B]
    # chunk k rows -> [p, (k, m)] so setup16[:, 128k:128k+128] is lhsT chunk k
    hidT2p = hidT2.reshape(8, 128, 128).transpose(1, 0, 2).reshape(128, 1024)
    bvT = np.asarray(b, dtype=np.float32).reshape(8, 128).T  # [128, 8]
    setup16 = np.ascontiguousarray(
        np.concatenate([hidT2p, bvT], axis=1).astype(np.float16)
    )
    setup32 = _IDENT
    in_maps = []
    for i in range(NCORES):
        shard = (
            np.asarray(encoder_outputs[i * LS : (i + 1) * LS], dtype=np.float32)
            .astype(np.float16)
            .reshape(LS * B, H)
        )
        in_maps.append(
            {
                "setup16": setup16,
                "setup32": setup32,
                "enc": np.ascontiguousarray(shard),
                "w": W16,
            }
        )
    return in_maps


def kernel(hidden, encoder_outputs, W, b):
    nc = _get_nc()
    in_maps = make_in_maps(hidden, encoder_outputs, W, b)
    res = run_bass_kernel_spmd(nc, in_maps, core_ids=list(range(NCORES)))
    out = np.concatenate([res.results[i]["out"] for i in range(NCORES)], axis=1)
    return out[:, None, :].astype(np.float32)


# revision 24
# speedup vs baseline: 1.0436x; 1.0259x over previous
"""Trainium2 Bass kernel for nn_Attention (general-score attention energies +
softmax over the batch axis).

Math (reference):
    proj     = einsum('lbh,oh->lbo', enc, W) + b      # [L, B, H]
    energies = einsum('bh,lbh->bl', hidden, proj)     # [B, L]
    attn     = softmax(energies, axis=0)[:, None, :]  # [B, 1, L]

Algebraic rewrite used here:
    energies[b, l] = (hidden @ W)[b] . enc[l, b] + hidden[b] . b
This removes the O(L*B*H*H) projection matmul entirely; the kernel is a
memory-bound stream over enc with a tiny [B,H]x[H,H] matmul up front.

fp16 strategy: enc / W / hidden are cast to fp16 on the host (pure dtype
compression, all FLOPs stay on device; fp32 accumulation everywhere).
Measured end-to-end rel err vs the fp32 reference: ~1.8e-3 (gate: 2e-2).
This halves HBM traffic AND enables the DVE 2x_1P perf mode.

The dot-product stream is compute-floored by the 1024-wide row sums:
every reduce flavor (STT/tensor_scalar accum, tensor_reduce, bn_stats,
ACT ACTIVATE+accum) runs at 1 elem/cycle/lane (~1.2-1.3us per [128,1024]
block); only the plain fp16 tensor_tensor multiply has a 2x mode
(~570ns/block grouped). GpSimd cannot run the accum ops at all (walrus
engine check) and contends with DVE for SBUF ports, so the optimal
schedule uses DVE+ACT only:
  - 44 "A" blocks: DVE grouped TT multiply (in place over the streamed
    tile, vs a stride-0-broadcast view of u) + ScalarE ACTIVATE(Copy)
    with fused accum_out for the row sum.
  - 20 "D" blocks: one fused DVE scalar_tensor_tensor (1x) does
    multiply+sum in a single pass.
Both engines land at ~56us of stream work, overlapping the ~50us DMA.

Distribution: enc is sharded along L across 8 cores (128 l-values per
core). The softmax is over the batch axis (per l), so every core's
softmax is fully local -- no collectives. hidden / W / b are replicated.

Setup path: W fp16 in four 1 MB k-major quarter DMAs; the 16 PE matmuls
for u = hidden @ W run k-outer so each k-chunk is consumed as it lands.
hidden^T arrives pre-transposed and pair-duplicated so the matmul output
covers all 128 PSUM partitions. Softmax runs in two column halves so
the first half overlaps the stream; output leaves in two [64,64] DMAs.

Timing (HW, neuron-profile, core 0): 86-91 us cool, up to ~103 us when
the HAM activity throttle (50% util limit, engages ~20 us in) bites
harder on a thermally loaded device. fp32 baseline: 119-142 us.
Breakdown (cool): ~10 us NEFF/queue startup, stream start ~20 us
(W wire + PE chain; fp16 matmul streams at ~630ns/512 cols, ~3x the
bf16 rate), DVE+ACT balanced stream ~52-55 us each, ~5 us tail.
"""

import numpy as np

import concourse.bass as bass
import concourse.bacc as bacc
import concourse.tile as tile
from concourse import mybir
from concourse.bass_utils import run_bass_kernel_spmd

F32 = mybir.dt.float32
F16 = mybir.dt.float16

B = 64          # batch
H = 1024        # hidden dim
L = 1024        # enc_len
NCORES = 8
LS = L // NCORES            # 128 l-values per core
NBLK = LS * B // 128        # 64 [128, 1024] blocks per core
# blocks per DMA tile: small leading tiles so compute starts early
TILE_BLOCKS = [4, 4] + [8] * 7
assert sum(TILE_BLOCKS) == NBLK
# A-blocks (DVE mult + ACT reduce) per tile; the rest are fused-STT D-blocks
TILE_A = [3, 3, 5, 6, 5, 6, 5, 6, 5]   # 44 A / 20 D
MULT = mybir.AluOpType.mult
ADD = mybir.AluOpType.add


def build_program() -> bacc.Bacc:
    nc = bacc.Bacc(
        "TRN2", target_bir_lowering=False, debug=False, num_devices=NCORES
    )

    setup16_p = nc.declare_dram_parameter("setup16", [128, 1032], F16, isOutput=False)
    setup32_p = nc.declare_dram_parameter("setup32", [128, 128], F32, isOutput=False)
    enc_p = nc.declare_dram_parameter("enc", [LS * B, H], F16, isOutput=False)
    w_p = nc.declare_dram_parameter("w", [H, H], F16, isOutput=False)
    out_p = nc.declare_dram_parameter("out", [B, LS], F32, isOutput=True)

    # NOTE: must be built as bacc.Bacc + nc.compile() -- the staged walrus
    # rejects multi-wait instructions emitted by raw Bass+Tile; bacc
    # legalizes them.
    with tile.TileContext(nc) as tc:
        with (
            tc.tile_pool(name="const", bufs=1) as cp,
            tc.tile_pool(name="stream", bufs=6) as sp,
            tc.tile_pool(name="ps1", bufs=1, space="PSUM") as pp1,
            tc.tile_pool(name="psu", bufs=1, space="PSUM") as ppu,
        ):
            # ---- input DMAs (setup on the ACT ring so it does not queue
            # behind W/enc on the SP ring) ----
            setup16 = cp.tile([128, 1032], F16)
            setup32 = cp.tile([128, 128], F32)
            nc.scalar.dma_start(setup16[:], setup16_p.ap())
            nc.scalar.dma_start(setup32[:], setup32_p.ap())
            hT2 = setup16[:, 0:1024]      # chunk k at [:, 128k:128k+128]
            bvT = setup16[:, 1024:1032]
            idn = setup32

            # W as [o%128, (o//128, h)] fp16, four 1 MB k-major quarters so
            # the k-outer matmul chain consumes chunks as they land
            wt = cp.tile([128, 8 * H], F16)
            wt3 = wt[:].rearrange("p (k h) -> p k h", k=8)
            wsrc = w_p.ap().rearrange("(k p) h -> p k h", p=128)
            for kh in range(4):
                nc.sync.dma_start(
                    wt3[:, 2 * kh : 2 * kh + 2, :],
                    wsrc[:, 2 * kh : 2 * kh + 2, :],
                )

            # ---- u = hidden @ W on both b-halves ([128, 1024] fp32 PSUM),
            # k-outer so chunk k is consumed as soon as its quarter lands
            psum_u = ppu.tile([128, H], F32, tag="psum_u")
            u16 = cp.tile([128, H], F16)
            for k in range(8):
                for n in range(2):
                    nc.tensor.matmul(
                        psum_u[:, 512 * n : 512 * (n + 1)],
                        lhsT=hT2[:, 128 * k : 128 * (k + 1)],
                        rhs=wt[:, 1024 * k + 512 * n : 1024 * k + 512 * n + 512],
                        start=(k == 0),
                        stop=(k == 7),
                    )
            # one PSUM->SBUF half-copy on each engine, in parallel
            nc.scalar.copy(u16[:, 0:512], psum_u[:, 0:512])
            nc.vector.tensor_copy(u16[:, 512:1024], psum_u[:, 512:1024])
            u16b = u16[:].rearrange("p (x h) -> p x h", x=1)

            # ---- c[b] = hidden[b] . bvec (both b-halves via dup'd hT2) ----
            psum_c = ppu.tile([128, 1], F32, tag="psum_c")
            for k in range(8):
                nc.tensor.matmul(
                    psum_c[:],
                    lhsT=hT2[:, 128 * k : 128 * (k + 1)],
                    rhs=bvT[:, k : k + 1],
                    start=(k == 0),
                    stop=(k == 7),
                )
            c2 = cp.tile([128, 1], F32)
            nc.scalar.copy(c2[:], psum_c[:])

            # ---- main stream ----
            enc_flat = enc_p.ap()  # [8192, 1024] fp16
            ecols = cp.tile([128, NBLK], F32)  # [128, 64]
            ecols2 = cp.tile([128, NBLK], F32)
            out_sb = cp.tile([B, LS], F32)
            ov = out_sb[:].rearrange("b (l two) -> b two l", two=2)

            def softmax_half(hf):
                # bias, then softmax over b for columns [32hf, 32hf+32)
                # (l-values [64hf, 64hf+64)), both l-parities
                cs = slice(32 * hf, 32 * hf + 32)
                nc.vector.tensor_scalar_add(
                    ecols2[:, cs], ecols[:, cs], c2[:, 0:1]
                )
                for lp in range(2):
                    psum_t = pp1.tile([32, B], F32, tag=f"pt{lp}")
                    nc.tensor.transpose(
                        psum_t[:],
                        ecols2[lp * B : (lp + 1) * B, cs],
                        idn[lp * B : (lp + 1) * B, lp * B : (lp + 1) * B],
                    )
                    negm = cp.tile([32, 1], F32, tag=f"negm{lp}")
                    nc.vector.tensor_reduce(
                        out=negm[:],
                        in_=psum_t[:],
                        axis=mybir.AxisListType.X,
                        op=mybir.AluOpType.max,
                        negate=True,
                    )
                    pexp = cp.tile([32, B], F32, tag=f"pexp{lp}")
                    ssum = cp.tile([32, 1], F32, tag=f"ssum{lp}")
                    nc.scalar.activation(
                        pexp[:],
                        psum_t[:],
                        mybir.ActivationFunctionType.Exp,
                        bias=negm[:, 0:1],
                        scale=1.0,
                        accum_out=ssum[:],
                    )
                    rs = cp.tile([32, 1], F32, tag=f"rs{lp}")
                    nc.vector.reciprocal(rs[:], ssum[:])
                    attn = cp.tile([32, B], F32, tag=f"attn{lp}")
                    nc.vector.tensor_scalar_mul(attn[:], pexp[:], rs[:, 0:1])
                    psum_o = pp1.tile([B, 32], F32, tag="po")
                    nc.tensor.transpose(psum_o[:], attn[:], idn[0:32, 0:32])
                    nc.vector.tensor_copy(ov[:, lp, cs], psum_o[:])
                # contiguous l-range [64hf, 64hf+64) covers both parities
                nc.sync.dma_start(
                    out_p.ap()[:, 64 * hf : 64 * hf + 64],
                    out_sb[:, 64 * hf : 64 * hf + 64],
                )

            c0 = 0
            for t, nq in enumerate(TILE_BLOCKS):
                na = TILE_A[t]
                et = sp.tile([128, 8 * H], F16, tag="et")
                src = enc_flat[128 * c0 : 128 * (c0 + nq)].rearrange(
                    "(q p) h -> p q h", p=128
                )
                nc.sync.dma_start(
                    et[:, 0 : nq * H].rearrange("p (q h) -> p q h", q=nq), src
                )
                # A-prefix: one grouped DVE multiply (fp16 2x, in place)...
                nc.vector.tensor_tensor(
                    out=et[:, 0 : na * H].rearrange("p (g h) -> p g h", g=na),
                    in0=et[:, 0 : na * H].rearrange("p (g h) -> p g h", g=na),
                    in1=u16b.broadcast_to((128, na, H)),
                    op=MULT,
                )
                # ...then per-block ACT row sums
                for q in range(na):
                    c = c0 + q
                    blk = et[:, H * q : H * (q + 1)]
                    nc.scalar.activation(
                        blk,
                        blk,
                        mybir.ActivationFunctionType.Copy,
                        bias=0.0,
                        scale=1.0,
                        accum_out=ecols[:, c : c + 1],
                    )
                # D-suffix: fused multiply+sum on DVE (1x STT)
                for q in range(na, nq):
                    c = c0 + q
                    blk = et[:, H * q : H * (q + 1)]
                    nc.vector.scalar_tensor_tensor(
                        out=blk,
                        in0=blk,
                        scalar=1.0,
                        in1=u16[:],
                        op0=MULT,
                        op1=MULT,
                        accum_out=ecols[:, c : c + 1],
                    )
                c0 += nq
                if c0 == 32:
                    softmax_half(0)
            softmax_half(1)

    nc.compile()
    return nc


_IDENT = np.eye(128, dtype=np.float32)
_NC_CACHE = []


def _get_nc() -> bacc.Bacc:
    if not _NC_CACHE:
        _NC_CACHE.append(build_program())
    return _NC_CACHE[0]


def make_in_maps(hidden, encoder_outputs, W, b):
    # host side does only layout transforms + fp16 dtype compression; all
    # FLOPs stay on device (fp32 accumulation)
    hidden = np.asarray(hidden, dtype=np.float32)
    W16 = np.ascontiguousarray(np.asarray(W, dtype=np.float32).astype(np.float16))
    hidT2 = np.concatenate([hidden.T, hidden.T], axis=1)  # [H, 2B]
    # chunk k rows -> [p, (k, m)] so setup16[:, 128k:128k+128] is lhsT chunk k
    hidT2p = hidT2.reshape(8, 128, 128).transpose(1, 0, 2).reshape(128, 1024)
    bvT = np.asarray(b, dtype=np.float32).reshape(8, 128).T  # [128, 8]
    setup16 = np.ascontiguousarray(
        np.concatenate([hidT2p, bvT], axis=1).astype(np.float16)
    )
    setup32 = _IDENT
    in_maps = []
    for i in range(NCORES):
        shard = (
            np.asarray(encoder_outputs[i * LS : (i + 1) * LS], dtype=np.float32)
            .astype(np.float16)
            .reshape(LS * B, H)
        )
        in_maps.append(
            {
                "setup16": setup16,
                "setup32": setup32,
                "enc": np.ascontiguousarray(shard),
                "w": W16,
            }
        )
    return in_maps


def kernel(hidden, encoder_outputs, W, b):
    nc = _get_nc()
    in_maps = make_in_maps(hidden, encoder_outputs, W, b)
    res = run_bass_kernel_spmd(nc, in_maps, core_ids=list(range(NCORES)))
    out = np.concatenate([res.results[i]["out"] for i in range(NCORES)], axis=1)
    return out[:, None, :].astype(np.float32)
